# revision 6
# baseline (speedup 1.0000x reference)
"""Trainium2 Bass kernel for nn_MultiHeadedAttention (B=8, S=1024, D=1024, H=16).

Sharding: data-parallel over batch across 8 NeuronCores (1 batch element/core).
Returns (output, attn) like the reference.
"""
import sys

for p in ("/opt/trn_rl_repo", "/opt/trn_rl_repo/concourse"):
    if p not in sys.path:
        sys.path.insert(0, p)

import numpy as np

B, S, D, H = 8, 1024, 1024, 16
HD = D // H          # 64
P = 128              # partitions
NCH = D // P         # 8 chunks
VW = H * (HD + 1)    # 1040: V_ext row width per j-chunk

_CACHED = {}
_name_counters = {}


def _tn(base):
    i = _name_counters.get(base, 0)
    _name_counters[base] = i + 1
    return f"{base}_{i}"


def _build():
    import concourse.bass as bass
    import concourse.mybir as mybir
    import concourse.tile as tile
    from concourse import bacc
    from concourse.masks import make_identity
    from contextlib import ExitStack

    f32 = mybir.dt.float32
    f32r = mybir.dt.float32r
    bf16 = mybir.dt.bfloat16
    u8 = mybir.dt.uint8
    Exp = mybir.ActivationFunctionType.Exp
    mult = mybir.AluOpType.mult
    add = mybir.AluOpType.add

    nc = bacc.Bacc("TRN2", target_bir_lowering=False)

    q_in = nc.dram_tensor("q", [S, D], f32, kind="ExternalInput")
    k_in = nc.dram_tensor("k", [S, D], f32, kind="ExternalInput")
    v_in = nc.dram_tensor("v", [S, D], f32, kind="ExternalInput")
    m_in = nc.dram_tensor("m", [S, S], u8, kind="ExternalInput")
    wk_in = nc.dram_tensor("Wk", [D, D], f32, kind="ExternalInput")
    wv_in = nc.dram_tensor("Wv", [D, D], f32, kind="ExternalInput")
    wo_in = nc.dram_tensor("Wo", [D, D], f32, kind="ExternalInput")
    bk_in = nc.dram_tensor("bk", [D], f32, kind="ExternalInput")
    bv_in = nc.dram_tensor("bv", [D], f32, kind="ExternalInput")
    bo_in = nc.dram_tensor("bo", [D], f32, kind="ExternalInput")
    attn_out = nc.dram_tensor("attn", [H, S, S], f32, kind="ExternalOutput")
    out_out = nc.dram_tensor("out", [S, D], f32, kind="ExternalOutput")
    rbounce = nc.dram_tensor("rbounce", [H, S], f32)
    ctdram = nc.dram_tensor("ctdram", [NCH, P, S], f32r)   # ctxT spill

    def dram_ap(handle, offset, ap):
        return bass.AP(tensor=handle.ap().tensor, offset=offset, ap=ap)

    with tile.TileContext(nc) as tc, ExitStack() as ctx:
        const = ctx.enter_context(tc.tile_pool(name="const", bufs=1))

        idf32 = const.tile([P, P], f32)
        make_identity(nc, idf32)
        idbf = const.tile([P, P], bf16)
        nc.gpsimd.memset(idbf, 0.0)
        nc.gpsimd.affine_select(out=idbf, in_=idbf, compare_op=mybir.AluOpType.not_equal,
                                fill=1.0, base=0, pattern=[[-1, P]], channel_multiplier=1)
        negid = const.tile([P, P], bf16)
        nc.gpsimd.memset(negid, 0.0)
        nc.gpsimd.affine_select(out=negid, in_=negid, compare_op=mybir.AluOpType.not_equal,
                                fill=-1e18, base=0, pattern=[[-1, P]], channel_multiplier=1)

        bk_cols = const.tile([P, NCH], f32)
        nc.sync.dma_start(out=bk_cols, in_=dram_ap(bk_in, 0, [[1, P], [P, NCH]]))
        bv_cols = const.tile([P, NCH], f32)
        nc.sync.dma_start(out=bv_cols, in_=dram_ap(bv_in, 0, [[1, P], [P, NCH]]))
        bvb = const.tile([P, D], f32)
        nc.gpsimd.dma_start(out=bvb, in_=dram_ap(bv_in, 0, [[0, P], [1, D]]))
        bob = const.tile([P, D], f32)
        nc.gpsimd.dma_start(out=bob, in_=dram_ap(bo_in, 0, [[0, P], [1, D]]))

        bigctx = ExitStack()
        big = bigctx.enter_context(tc.tile_pool(name="big", bufs=1))
        QT = big.tile([P, NCH * S], f32r, tag="QT", name="QT")
        KT = big.tile([P, NCH * S], f32r, tag="KT", name="KT")
        VX = big.tile([P, NCH * VW], f32r, tag="VX", name="VX")

        vx_ones = bass.AP(tensor=VX.tensor, offset=VX.offset + HD,
                          ap=[VX.ap[0], [VW, NCH], [HD + 1, H]])
        nc.vector.memset(vx_ones.bitcast(f32), 1.0)

        # ---------------- Phase B ----------------
        def transpose_into(dst_tile, src_dram, pspool, ident, evict):
            """dst col-block cc = transpose of src; dst free width S per block."""
            with tc.tile_pool(name=_tn("natp"), bufs=NCH) as natp:
                nat = []
                for rc in range(NCH):
                    t = natp.tile([P, S], f32, tag="nat", name=_tn("nat"))
                    nc.sync.dma_start(out=t, in_=src_dram[rc * P:(rc + 1) * P, :])
                    nat.append(t)
                for cc in range(NCH):
                    ps = pspool.tile([P, S], f32, tag="tps", name=_tn("tps"))
                    for rc in range(NCH):
                        nc.tensor.transpose(ps[:, rc * P:(rc + 1) * P],
                                            nat[rc][:, cc * P:(cc + 1) * P], ident)
                    evict(dst_tile[:, cc * S:(cc + 1) * S], ps, cc)

        def evict_copy(dst, ps, cc):
            nc.scalar.copy(dst, ps)

        def project(dst_tile, xT, wT, pspool, bias_cols=None, swap=False, vx=False):
            for oc in range(NCH):
                ps = pspool.tile([P, S], f32, tag="tps", name=_tn("pps"))
                for half in range(2):
                    sl = slice(half * 512, (half + 1) * 512)
                    for cc in range(NCH):
                        if swap:
                            lhs = xT[:, cc * S + oc * P: cc * S + (oc + 1) * P]
                            rhs = wT[:, cc * S:(cc + 1) * S][:, sl]
                        else:
                            lhs = wT[:, cc * S + oc * P: cc * S + (oc + 1) * P]
                            rhs = xT[:, cc * S:(cc + 1) * S][:, sl]
                        nc.tensor.matmul(ps[:, sl], lhs, rhs,
                                         start=(cc == 0), stop=(cc == NCH - 1))
                if vx:
                    dst = bass.AP(tensor=dst_tile.tensor,
                                  offset=dst_tile.offset + oc * VW,
                                  ap=[dst_tile.ap[0], [HD + 1, H], [1, HD]])
                    nc.vector.tensor_tensor(out=dst, in0=ps, in1=bvb, op=add)
                elif bias_cols is None:
                    nc.scalar.copy(dst_tile[:, oc * S:(oc + 1) * S], ps)
                else:
                    nc.vector.tensor_scalar_add(dst_tile[:, oc * S:(oc + 1) * S],
                                                ps, bias_cols[:, oc:oc + 1])

        with tc.tile_pool(name="bps", bufs=3, space="PSUM") as bps:
            with tc.tile_pool(name="wvp", bufs=1) as wvp:
                WvT = wvp.tile([P, NCH * S], f32r, tag="WvT", name="WvT")
                transpose_into(WvT, wv_in.ap(), bps, idf32, evict_copy)
                # q -> QT (uses Wv/bv per reference's no-cache branch)
                with tc.tile_pool(name="xqp", bufs=1) as xqp:
                    xT = xqp.tile([P, NCH * S], f32r, tag="xTq", name="xTq")
                    transpose_into(xT, q_in.ap(), bps, idf32, evict_copy)
                    project(QT, xT, WvT, bps, bias_cols=bv_cols)
                # v -> VX
                with tc.tile_pool(name="xvp", bufs=1) as xvp:
                    xT = xvp.tile([P, NCH * S], f32r, tag="xTv", name="xTv")
                    transpose_into(xT, v_in.ap(), bps, idf32, evict_copy)
                    project(VX, xT, WvT, bps, swap=True, vx=True)
            with tc.tile_pool(name="wkp", bufs=1) as wkp:
                WkT = wkp.tile([P, NCH * S], f32r, tag="WkT", name="WkT")
                transpose_into(WkT, wk_in.ap(), bps, idf32, evict_copy)
                with tc.tile_pool(name="xkp", bufs=1) as xkp:
                    xT = xkp.tile([P, NCH * S], f32r, tag="xTk", name="xTk")
                    transpose_into(xT, k_in.ap(), bps, idf32, evict_copy)
                    project(KT, xT, WkT, bps, bias_cols=bk_cols)


        # masks (allocated after projection transients are freed)
        maskp = bigctx.enter_context(tc.tile_pool(name="maskp", bufs=1))
        maskT = maskp.tile([P, NCH * S], bf16, tag="maskT", name="maskT")
        nmask = maskp.tile([P, NCH * S], bf16, tag="nmask", name="nmask")
        with tc.tile_pool(name="mps", bufs=2, space="PSUM") as mps, \
             tc.tile_pool(name="mtmp", bufs=1) as mp:
            mnat = mp.tile([P, NCH * S], bf16, tag="mnat", name="mnat")
            with tc.tile_pool(name="mu8p", bufs=2) as mup:
                for ic in range(NCH):
                    mu = mup.tile([P, S], u8, tag="mu8", name=_tn("mu8"))
                    nc.sync.dma_start(out=mu, in_=m_in.ap()[ic * P:(ic + 1) * P, :])
                    nc.vector.tensor_scalar(nmask[:, ic * S:(ic + 1) * S], mu,
                                            -1.0, 1.0, mult, add)
                    nc.vector.tensor_copy(mnat[:, ic * S:(ic + 1) * S], mu)
            for cc in range(NCH):
                psb = mps.tile([P, S], bf16, tag="mtps", name=_tn("tpsb"))
                for rc in range(NCH):
                    nc.tensor.transpose(psb[:, rc * P:(rc + 1) * P],
                                        mnat[:, rc * S + cc * P: rc * S + (cc + 1) * P],
                                        idbf)
                nc.vector.tensor_copy(maskT[:, cc * S:(cc + 1) * S], psb)

        # ---------------- Phase C ----------------
        with tc.tile_pool(name="cps", bufs=2, space="PSUM") as cps, \
             tc.tile_pool(name="avps", bufs=1, space="PSUM") as avps, \
             tc.tile_pool(name="rtps", bufs=1, space="PSUM") as rtps, \
             tc.tile_pool(name="ptp", bufs=3) as ptp, \
             tc.tile_pool(name="pij", bufs=4) as pij, \
             tc.tile_pool(name="smalls", bufs=2) as smalls, \
             tc.tile_pool(name="ctb", bufs=2) as ctbp:

            rT_tiles = {}

            def emit_T(h):
                hc, ro = h // 2, (h % 2) * HD
                av = avps.tile([P, S], f32, tag="av", name=_tn("av"))
                for jc in range(NCH):
                    ps = cps.tile([P, S], f32, tag="sc", name=_tn("scT"))
                    for ih in range(2):
                        sl = slice(ih * 512, (ih + 1) * 512)
                        nc.tensor.matmul(
                            ps[:, sl],
                            KT[ro:ro + HD, hc * S + jc * P: hc * S + (jc + 1) * P],
                            QT[ro:ro + HD, hc * S: (hc + 1) * S][:, sl],
                            start=True, stop=False)
                        nc.tensor.matmul(ps[:, sl], negid,
                                         maskT[:, jc * S:(jc + 1) * S][:, sl],
                                         start=False, stop=True)
                    pt = ptp.tile([P, S], f32r, tag="pt", name=_tn("pt"))
                    nc.scalar.activation(pt, ps, Exp, bias=0.0, scale=0.125)
                    for ih in range(2):
                        sl = slice(ih * 512, (ih + 1) * 512)
                        nc.tensor.matmul(
                            av[0:HD + 1, sl],
                            VX[:, jc * VW + h * (HD + 1): jc * VW + (h + 1) * (HD + 1)],
                            pt[:, sl],
                            start=(jc == 0), stop=(jc == NCH - 1))
                rr65 = smalls.tile([HD + 1, S], f32, tag="rr", name=_tn("rr"))
                nc.vector.reciprocal(rr65[HD:HD + 1, :], av[HD:HD + 1, :])
                nc.sync.dma_start(out=rbounce.ap()[h:h + 1, :], in_=rr65[HD:HD + 1, :])
                rb = smalls.tile([HD, S], f32, tag="rb", name=_tn("rb"))
                nc.gpsimd.dma_start(out=rb, in_=dram_ap(rbounce, h * S, [[0, HD], [1, S]]))
                ctb = ctbp.tile([HD, S], f32r, tag="ctb", name=_tn("ctb"))
                nc.vector.tensor_tensor(out=ctb, in0=av[0:HD, :], in1=rb, op=mult)
                nc.sync.dma_start(out=ctdram.ap()[hc, ro:ro + HD, :], in_=ctb)
                rr0 = smalls.tile([1, S], f32, tag="rr0", name=_tn("rr0"))
                nc.sync.dma_start(out=rr0, in_=rbounce.ap()[h:h + 1, :])
                rps = rtps.tile([P, NCH], f32, tag="rt", name=_tn("rt"))
                for ic in range(NCH):
                    nc.tensor.transpose(rps[:, ic:ic + 1], rr0[0:1, ic * P:(ic + 1) * P],
                                        idf32[0:1, 0:1])
                rT = smalls.tile([P, NCH], f32, tag="rTs", name=_tn("rTs"))
                nc.vector.tensor_copy(rT, rps)
                rT_tiles[h] = rT

            def emit_ij(h):
                hc, ro = h // 2, (h % 2) * HD
                rT = rT_tiles.pop(h)
                for ic in range(NCH):
                    ps = cps.tile([P, S], f32, tag="sc", name=_tn("scI"))
                    for jh in range(2):
                        sl = slice(jh * 512, (jh + 1) * 512)
                        nc.tensor.matmul(
                            ps[:, sl],
                            QT[ro:ro + HD, hc * S + ic * P: hc * S + (ic + 1) * P],
                            KT[ro:ro + HD, hc * S: (hc + 1) * S][:, sl],
                            start=True, stop=True)
                    pe = pij.tile([P, S], f32, tag="pe", name=_tn("pe"))
                    nc.scalar.activation(pe, ps, Exp, bias=0.0, scale=0.125)
                    nc.vector.scalar_tensor_tensor(pe, pe, rT[:, ic:ic + 1],
                                                   nmask[:, ic * S:(ic + 1) * S],
                                                   mult, mult)
                    nc.sync.dma_start(out=attn_out.ap()[h, ic * P:(ic + 1) * P, :],
                                      in_=pe)

            for step in range(H + 1):
                if step < H:
                    emit_T(step)
                if step >= 1:
                    emit_ij(step - 1)

        bigctx.close()

        # ---------------- Phase D ----------------
        with tc.tile_pool(name="dps", bufs=2, space="PSUM") as dps, \
             tc.tile_pool(name="wop", bufs=1) as wop, \
             tc.tile_pool(name="ctsp", bufs=1) as ctsp, \
             tc.tile_pool(name="ostg", bufs=2) as ostg:
            WoT = wop.tile([P, NCH * S], f32r, tag="WoT", name="WoT")
            transpose_into(WoT, wo_in.ap(), dps, idf32, evict_copy)
            CTs = ctsp.tile([P, NCH * S], f32r, tag="CTs", name="CTs")
            for cc in range(NCH):
                nc.sync.dma_start(out=CTs[:, cc * S:(cc + 1) * S],
                                  in_=ctdram.ap()[cc, :, :])
            for ic in range(NCH):
                ps = dps.tile([P, S], f32, tag="ops", name=_tn("ops"))
                for half in range(2):
                    sl = slice(half * 512, (half + 1) * 512)
                    for dc in range(NCH):
                        nc.tensor.matmul(
                            ps[:, sl],
                            CTs[:, dc * S + ic * P: dc * S + (ic + 1) * P],
                            WoT[:, dc * S:(dc + 1) * S][:, sl],
                            start=(dc == 0), stop=(dc == NCH - 1))
                ot = ostg.tile([P, S], f32, tag="ot", name=_tn("ot"))
                nc.vector.tensor_tensor(out=ot, in0=ps, in1=bob, op=add)
                nc.sync.dma_start(out=out_out.ap()[ic * P:(ic + 1) * P, :], in_=ot)

    nc.finalize()
    return nc


def kernel(key, value, query, mask, Wk, bk, Wv, bv, Wo, bo):
    from concourse.bass_utils import run_bass_kernel_spmd

    if "nc" not in _CACHED:
        _CACHED["nc"] = _build()
    nc = _CACHED["nc"]

    mask_u8 = np.ascontiguousarray(mask).view(np.uint8)
    in_maps = []
    for b in range(B):
        in_maps.append({
            "q": np.ascontiguousarray(query[b]),
            "k": np.ascontiguousarray(key[b]),
            "v": np.ascontiguousarray(value[b]),
            "m": mask_u8[b],
            "Wk": np.ascontiguousarray(Wk), "bk": np.ascontiguousarray(bk),
            "Wv": np.ascontiguousarray(Wv), "bv": np.ascontiguousarray(bv),
            "Wo": np.ascontiguousarray(Wo), "bo": np.ascontiguousarray(bo),
        })
    res = run_bass_kernel_spmd(nc, in_maps, core_ids=list(range(B)))
    _CACHED["last_result"] = res
    output = np.stack([res.results[b]["out"] for b in range(B)])
    attn = np.stack([res.results[b]["attn"] for b in range(B)])
    return output, attn
